# revision 1
# baseline (speedup 1.0000x reference)
"""Trainium2 Bass kernel for nn_ActorNetwork (gnn_message_passing).

Mathematical collapse (verified vs reference to ~2.5e-8 rel): the reference
broadcasts edge_index as ``broadcast_to(ei[None], (B,2,E)).reshape(2,-1)``,
making row == col elementwise -> every edge is a self-loop and the GCN
normalization cancels exactly: ``gcn_conv(x, W, b) == x @ W + b``.  The
network is two dense layers + softmax over nodes, plus a per-(node,k)
2-layer MLP + softmax over k.  ``edge_index`` never ships to the device.
Scalar biases bfc / bc2 are constant softmax shifts and cancel.

Device strategy (data-parallel over batch, core i = graphs 2i, 2i+1):
 - Chunk remap: group g holds chunks {g, 5+g, 10+g, 15+g} at rows j=0..3,
   so graph membership = j//2 for EVERY group; the node-softmax scale
   1/S folds into one bstb vector used as the mbp lhsT.
 - Interleaved per-group pipeline: node path then col path per group,
   xg/cl DMAs emitted in-loop on the sync ring; consts ship via
   gpsimd(SWDGE)/scalar rings in parallel.
 - Pair packing: l4p/el/d4p land on partitions 64-67 (even groups) or
   96-99 (odd groups, via tile_position col 96), so ONE [36,500] ACT op
   covers a pair for lnD/dinv and one GpSimd multiply covers m4 (the
   garbage middle rows 68-95 are never in any consumer AP; accs rows are
   pre-zeroed so the S matmul never multiplies junk).
 - The softmax sum S is ready after group 4's NODE path, so the S chain
   plus finalize (mbp -> ot -> store) for groups 0-3 is interspersed
   between group 4's col-path stages; only group 4's finalize trails.
 - Inputs fp8e4m3, intermediates bf16, accumulation f32, output bf16.
"""

import numpy as np

B, N, F, K, FC = 16, 5000, 128, 10, 32
NCORES = 8
GPC = B // NCORES          # graphs per core = 2
M = GPC * N                # nodes per core = 10000
CH = 500                   # chunk size (nodes)
NCHUNK = M // CH           # 20
GRP = 4                    # chunks per group
NGRP = NCHUNK // GRP       # 5
XW = GRP * CH              # node-feature cols per group = 2000
CW = 5000                  # col-blob cols per group: 2000+2000+1000

_W8, _WBF, _W32 = {}, {}, {}
def _mk(dct, *spec):
    off = 0
    for name, width in spec:
        dct[name] = (off, off + width)
        off += width
    return off
_NW8 = _mk(_W8, ("w1p", 32), ("wc1b", 64), ("wc1b2p", 64), ("pad8", 352))
_NWB = _mk(_WBF, ("w2b", 64), ("wfcb", 4), ("wc2a", 32), ("wc2b", 32),
           ("kmask", 4), ("bmask", 128))
_NW32 = _mk(_W32, ("smsk2", 2), ("smap2", 4),
            ("b1r", 1), ("b2r", 1), ("bc1r", 1))


def _pack_consts(W1, b1, W2, b2, Wfc, Wc1, bc1, Wc2):
    import ml_dtypes
    w8 = np.zeros((128, _NW8), np.float32)
    wb = np.zeros((128, _NWB), np.float32)
    w3 = np.zeros((128, _NW32), np.float32)

    lo = _W8["w1p"][0]
    w8[:, lo:lo + 16] = W1
    lo = _W8["wc1b"][0]
    for a in range(4):
        w8[32 * a:32 * a + 32, lo + 16 * a:lo + 16 * a + 16] = Wc1
    lo = _W8["wc1b2p"][0]
    for s in range(2):
        for t in range(2):
            r = 64 * s + 32 * t
            w8[r:r + 32, lo + 32 * s + 16 * t:lo + 32 * s + 16 * t + 16] = Wc1

    lo = _WBF["w2b"][0]
    for j in range(4):
        wb[32 * j:32 * j + 16, lo + 16 * j:lo + 16 * j + 16] = W2
    lo = _WBF["wfcb"][0]
    for j in range(4):
        wb[16 * j:16 * j + 16, lo + j] = Wfc[:, 0]
    lo = _WBF["wc2a"][0]
    for k in range(8):
        wb[16 * k:16 * k + 16, lo + k] = Wc2[:, 0]
    lo = _WBF["wc2b"][0]
    for j in range(4):
        for t in range(2):
            r = 32 * j + 16 * t
            wb[r:r + 16, lo + 8 + t] = Wc2[:, 0]
    lo = _WBF["kmask"][0]
    for j in range(4):
        wb[32 * j:32 * j + 10, lo + j] = 1.0
    lo = _WBF["bmask"][0]
    for j in range(4):
        wb[64 + j, lo + 32 * j:lo + 32 * j + 10] = 1.0
        wb[96 + j, lo + 32 * j:lo + 32 * j + 10] = 1.0

    lo = _W32["smsk2"][0]
    for j in range(4):
        w3[64 + j, lo + j // 2] = 1.0
        w3[96 + j, lo + j // 2] = 1.0
    lo = _W32["smap2"][0]
    for j in range(4):
        w3[j // 2, lo + j] = 1.0
    lo = _W32["b1r"][0]
    for j in range(4):
        w3[32 * j:32 * j + 16, lo] = b1
    lo = _W32["b2r"][0]
    for j in range(4):
        w3[16 * j:16 * j + 16, lo] = b2
    lo = _W32["bc1r"][0]
    for s in range(8):
        w3[16 * s:16 * s + 16, lo] = bc1
    return (w8.astype(ml_dtypes.float8_e4m3), wb.astype(ml_dtypes.bfloat16),
            w3)


_CACHED = None


def _build():
    from contextlib import ExitStack

    import concourse.tile as tile
    from concourse import bacc, mybir

    f32 = mybir.dt.float32
    bf16 = mybir.dt.bfloat16
    f8 = mybir.dt.float8e4
    AF = mybir.ActivationFunctionType
    ALU = mybir.AluOpType

    import concourse.bacc as bacc_mod
    _orig_gat = bacc_mod.get_activation_tables

    def _gat_one_set(arch):
        t = _orig_gat(arch)
        if "natural_log_exp_and_others" not in t:
            return t
        return {k: (v if k == "natural_log_exp_and_others" else set())
                for k, v in t.items()}

    bacc_mod.get_activation_tables = _gat_one_set

    nc = bacc.Bacc("TRN2", target_bir_lowering=False, debug=False,
                   num_devices=NCORES)

    xt_p = nc.dram_tensor("xt", [128, NGRP * XW], f8, kind="ExternalInput").ap()
    cl_p = nc.dram_tensor("cl", [128, NGRP * CW], f8, kind="ExternalInput").ap()
    w8_p = nc.dram_tensor("w8", [128, _NW8], f8, kind="ExternalInput").ap()
    wb_p = nc.dram_tensor("wb", [128, _NWB], bf16, kind="ExternalInput").ap()
    w3_p = nc.dram_tensor("w3", [128, _NW32], f32, kind="ExternalInput").ap()
    out_p = nc.dram_tensor("out", [128, NGRP * CH], bf16,
                       kind="ExternalOutput").ap()

    with tile.TileContext(nc) as tc, ExitStack() as ctx:
        wpool = ctx.enter_context(tc.tile_pool(name="wc", bufs=1))
        wt8 = wpool.tile([128, _NW8], f8, tag="wt8")
        wtb = wpool.tile([128, _NWB], bf16, tag="wtb")
        wt3 = wpool.tile([128, _NW32], f32, tag="wt3")

        def w8s(name, rows=128):
            lo, hi = _W8[name]
            return wt8[0:rows, lo:hi]

        def wbs(name, rows=128):
            lo, hi = _WBF[name]
            return wtb[0:rows, lo:hi]

        def w3s(name, rows=128):
            lo, hi = _W32[name]
            return wt3[0:rows, lo:hi]

        sb = ctx.enter_context(tc.tile_pool(name="sb", bufs=2))
        xp = ctx.enter_context(tc.tile_pool(name="xp", bufs=NGRP))
        cp = ctx.enter_context(tc.tile_pool(name="cp", bufs=NGRP))
        h1sp = ctx.enter_context(tc.tile_pool(name="h1sp", bufs=2))
        elp = ctx.enter_context(tc.tile_pool(name="elp", bufs=NGRP))
        eyp = ctx.enter_context(tc.tile_pool(name="eyp", bufs=NGRP))
        dvp = ctx.enter_context(tc.tile_pool(name="dvp", bufs=2))
        m4p = ctx.enter_context(tc.tile_pool(name="m4p", bufs=NGRP))
        accp = ctx.enter_context(tc.tile_pool(name="accp", bufs=1))
        outp = ctx.enter_context(tc.tile_pool(name="outp", bufs=NGRP))
        accs = accp.tile([128, 3], f32)

        # consts on parallel rings: w8+w3 via gpsimd (SWDGE), wb via scalar
        nc.gpsimd.dma_start(out=wt8[:], in_=w8_p[:])
        nc.gpsimd.dma_start(out=wt3[:], in_=w3_p[:])
        nc.scalar.dma_start(out=wtb[:], in_=wb_p[:])
        nc.vector.memset(accs[64:100, :], 0.0)
        # pre-load the ACT spline table set with no DMA dependency
        warm = sb.tile([1, 2], f32, tag="warm")
        nc.vector.memset(warm[:, 0:1], 0.0)
        nc.scalar.activation(warm[:, 1:2], warm[:, 0:1], AF.Exp)

        fpool = ctx.enter_context(tc.tile_pool(name="finp", bufs=1,
                                               space="PSUM"))
        ft = fpool.tile([128, 512], f32, tag="fin", name="fin")
        sp = ft[0:2, 504:505]
        s4p = ft[64:68, 508:509]

        el_tiles = {}

        # ======== main loop: node + col per group (interleaved) ========
        with tc.tile_pool(name="h1pp", bufs=1, space="PSUM") as h1pp, \
             tc.tile_pool(name="h2lp", bufs=1, space="PSUM") as h2lp, \
             tc.tile_pool(name="h01pp", bufs=2, space="PSUM") as h01pp, \
             tc.tile_pool(name="h2cpp", bufs=1, space="PSUM") as h2cpp, \
             tc.tile_pool(name="ypp", bufs=1, space="PSUM") as ypp, \
             tc.tile_pool(name="d4pp", bufs=1, space="PSUM") as d4pp:
            ey_tiles, m4_tiles = [], []
            d4_tiles, el_tiles = {}, {}
            xgts, colts = [], []
            fin_env = {}

            def fin(cc):
                ro2 = 64 if cc % 2 == 0 else 96
                mbp = ft[:, 0:CH]
                nc.tensor.matmul(mbp, lhsT=fin_env["bstb"][ro2:ro2 + 4, :],
                                 rhs=m4_tiles[cc // 2][ro2:ro2 + 4, :],
                                 start=True, stop=True,
                                 tile_position=(ro2, 0),
                                 skip_group_check=True)
                ot = outp.tile([128, CH], bf16, tag="ot")
                nc.vector.tensor_mul(ot[:], ey_tiles[cc][:], mbp)
                nc.sync.dma_start(out=out_p[:, CH * cc:CH * (cc + 1)],
                                  in_=ot[:])
            for c in range(NGRP):
                xg = xp.tile([128, XW], f8, tag="xg", name="xg")
                nc.sync.dma_start(out=xg[:],
                                  in_=xt_p[:, XW * c:XW * (c + 1)])
                xgts.append(xg)
                cl2 = cp.tile([128, CW], f8, tag="cl", name="cl")
                nc.sync.dma_start(out=cl2[:],
                                  in_=cl_p[:, CW * c:CW * (c + 1)])
                colts.append(cl2)
                # --- node path ---
                h1p = h1pp.tile([128, 512], f32, tag="h1p",
                                name="h1p")[:, 0:CH]
                for j in range(GRP):
                    nc.tensor.matmul(h1p[32 * j:32 * j + 32, :],
                                     lhsT=w8s("w1p"),
                                     rhs=xg[:, CH * j:CH * (j + 1)],
                                     start=True, stop=True,
                                     tile_position=(0, 32 * j),
                                     skip_group_check=True)
                h1st = h1sp.tile([128, 512], bf16, tag="h1s")
                nc.vector.tensor_scalar(h1st[:, 0:CH], h1p[:],
                                        w3s("b1r"), 0.0, ALU.add, ALU.max)
                h2lt = h2lp.tile([128, 512], f32, tag="h2l", name="h2l")
                nc.tensor.matmul(h2lt[0:64, 0:CH], lhsT=wbs("w2b"),
                                 rhs=h1st[:, 0:CH],
                                 start=True, stop=True,
                                 skip_group_check=True)
                h2st = sb.tile([64, 512], bf16, tag="h2s")
                nc.scalar.activation(h2st[:, 0:CH], h2lt[0:64, 0:CH],
                                     AF.Relu, bias=w3s("b2r", 64))
                pc = c // 2
                ro = 64 if c % 2 == 0 else 96
                nc.tensor.matmul(h2lt[ro:ro + 4, 0:CH],
                                 lhsT=wbs("wfcb", 64),
                                 rhs=h2st[:, 0:CH],
                                 start=True, stop=True,
                                 tile_position=(0, ro),
                                 skip_group_check=True)
                if c % 2 == 0:
                    elt = elp.tile([128, CH], bf16, tag="el")
                    el_tiles[pc] = elt
                else:
                    elt = el_tiles[pc]
                nc.scalar.activation(elt[ro:ro + 4, :], h2lt[ro:ro + 4, 0:CH],
                                     AF.Exp,
                                     accum_out=accs[ro:ro + 4, pc:pc + 1])
                if c % 2 == 1 or c == NGRP - 1:
                    nc.tensor.matmul(sp, lhsT=w3s("smsk2")[64:100, :],
                                     rhs=accs[64:100, pc:pc + 1],
                                     start=(pc == 0), stop=(c == NGRP - 1),
                                     tile_position=(64, 0),
                                     skip_group_check=True)

                # --- col path ---
                if c == NGRP - 1:
                    # S chain: everything below depends only on node paths
                    sinv = sb.tile([2, 1], f32, tag="sinv")
                    nc.vector.reciprocal(sinv[:], sp)
                    nc.tensor.matmul(s4p, lhsT=w3s("smap2", 2), rhs=sinv[:],
                                     start=True, stop=True,
                                     tile_position=(0, 64),
                                     skip_group_check=True)
                    s4p2 = ft[96:100, 508:509]
                    nc.tensor.matmul(s4p2, lhsT=w3s("smap2", 2), rhs=sinv[:],
                                     start=True, stop=True,
                                     tile_position=(0, 96),
                                     skip_group_check=True)
                    s4s = sb.tile([128, 1], f32, tag="s4s")
                    nc.vector.tensor_copy(s4s[64:68, :], s4p)
                    nc.vector.tensor_copy(s4s[96:100, :], s4p2)
                    bstb = sb.tile([128, 128], bf16, tag="bstb")
                    blo, _bhi = _WBF["bmask"]
                    nc.vector.tensor_scalar_mul(bstb[64:68, :],
                                                wtb[64:68, blo:blo + 128],
                                                s4s[64:68, :])
                    nc.vector.tensor_scalar_mul(bstb[96:100, :],
                                                wtb[96:100, blo:blo + 128],
                                                s4s[96:100, :])
                    fin_env["bstb"] = bstb
                    fin(0)
                c0 = cl2[:, 0:2000]
                c1 = cl2[:, 2000:4000]
                c2t = cl2[:, 4000:5000]
                h2cp = h2cpp.tile([128, 512], f32, tag="h2cp",
                                  name="h2cp")[:, 0:CH]
                for p in range(2):
                    nc.tensor.matmul(h2cp[64 * p:64 * p + 64, :],
                                     lhsT=w8s("wc1b2p"),
                                     rhs=c2t[:, CH * p:CH * (p + 1)],
                                     start=True, stop=True,
                                     tile_position=(0, 64 * p))
                h01s_r = []
                for j in range(2):
                    cs = slice(CH * j, CH * (j + 1))
                    h01p = h01pp.tile([128, 512], f32, tag="h01p",
                                      name="h01p")[:, 0:CH]
                    nc.tensor.matmul(h01p[0:64, :], lhsT=w8s("wc1b"),
                                     rhs=c0[:, cs], start=True, stop=True)
                    nc.tensor.matmul(h01p[64:128, :], lhsT=w8s("wc1b"),
                                     rhs=c1[:, cs], start=True, stop=True,
                                     tile_position=(0, 64))
                    hs = sb.tile([128, CH], bf16, tag=f"h01s{j % 2}")
                    if j == 3:
                        nc.scalar.activation(hs[:], h01p[:], AF.Relu,
                                             bias=w3s("bc1r"))
                    else:
                        nc.vector.tensor_scalar(hs[:], h01p[:], w3s("bc1r"),
                                                0.0, ALU.add, ALU.max)
                    h01s_r.append(hs)
                    if c == NGRP - 1 and j == 1:
                        fin(1)
                for j in range(2, GRP):
                    cs = slice(CH * j, CH * (j + 1))
                    h01p = h01pp.tile([128, 512], f32, tag="h01p",
                                      name="h01p")[:, 0:CH]
                    nc.tensor.matmul(h01p[0:64, :], lhsT=w8s("wc1b"),
                                     rhs=c0[:, cs], start=True, stop=True)
                    nc.tensor.matmul(h01p[64:128, :], lhsT=w8s("wc1b"),
                                     rhs=c1[:, cs], start=True, stop=True,
                                     tile_position=(0, 64))
                    hs = sb.tile([128, CH], bf16, tag=f"h01s{j % 2}")
                    if j == 3:
                        nc.scalar.activation(hs[:], h01p[:], AF.Relu,
                                             bias=w3s("bc1r"))
                    else:
                        nc.vector.tensor_scalar(hs[:], h01p[:], w3s("bc1r"),
                                                0.0, ALU.add, ALU.max)
                    h01s_r.append(hs)
                    if c == NGRP - 1 and j == 3:
                        fin(2)
                h2cs = sb.tile([128, CH], bf16, tag="h2cs")
                nc.scalar.activation(h2cs[:], h2cp[:], AF.Relu,
                                     bias=w3s("bc1r"))
                yp = ypp.tile([128, 512], f32, tag="yp", name="yp")[:, 0:CH]
                for j in range(GRP):
                    nc.tensor.matmul(yp[32 * j:32 * j + 32, :],
                                     lhsT=wbs("wc2a"), rhs=h01s_r[j][:],
                                     start=True, stop=False,
                                     skip_group_check=True,
                                     tile_position=(0, 32 * j))
                for j in range(GRP):
                    nc.tensor.matmul(yp[32 * j:32 * j + 32, :],
                                     lhsT=wtb[32 * j:32 * j + 32,
                                              slice(*_WBF["wc2b"])],
                                     rhs=h2cs[32 * j:32 * j + 32, :],
                                     start=False, stop=True,
                                     skip_group_check=True,
                                     tile_position=(32 * j, 32 * j))
                if c == NGRP - 1:
                    fin(3)
                ey = eyp.tile([128, CH], bf16, tag="ey")
                nc.scalar.activation(ey[:], yp[:], AF.Exp)
                ey_tiles.append(ey)
                if c % 2 == 0:
                    d4t = d4pp.tile([128, 512], f32, tag="d4p", name="d4p")
                    d4_tiles[pc] = d4t
                else:
                    d4t = d4_tiles[pc]
                nc.tensor.matmul(d4t[ro:ro + 4, 0:CH], lhsT=wbs("kmask"),
                                 rhs=ey[:],
                                 start=True, stop=True,
                                 tile_position=(0, ro),
                                 skip_group_check=True)
                if c % 2 == 1 or c == NGRP - 1:
                    hi = 100 if c % 2 == 1 else 68
                    lnD = sb.tile([128, CH], f32, tag="lnD")
                    nc.scalar.activation(lnD[64:hi, :], d4t[64:hi, 0:CH],
                                         AF.Ln)
                    dinv = dvp.tile([128, CH], f32, tag="dinv")
                    nc.scalar.activation(dinv[64:hi, :], lnD[64:hi, :],
                                         AF.Exp, scale=-1.0)
                    m4 = m4p.tile([128, CH], bf16, tag="m4")
                    nc.gpsimd.tensor_mul(m4[64:hi, :],
                                         el_tiles[pc][64:hi, :],
                                         dinv[64:hi, :])
                    m4_tiles.append(m4)

            # last group's own finalize
            fin(NGRP - 1)

    nc.compile()
    bacc_mod.get_activation_tables = _orig_gat
    return nc


def _get_compiled():
    global _CACHED
    if _CACHED is None:
        _CACHED = _build()
    return _CACHED


def _prep_inputs(node_features, col_features, W1, b1, W2, b2, Wfc,
                 Wc1, bc1, Wc2):
    import ml_dtypes
    f8 = ml_dtypes.float8_e4m3
    nf = np.asarray(node_features, np.float32)
    cf = np.asarray(col_features, np.float32)
    xt = np.ascontiguousarray(
        nf.reshape(NCORES, GPC, N, F).transpose(0, 3, 1, 2)
        .reshape(NCORES, F, M)).astype(f8)
    ctf = np.ascontiguousarray(
        cf.reshape(NCORES, GPC, N, K, FC).transpose(0, 3, 4, 1, 2)
        .reshape(NCORES, K * FC, M)).astype(f8)
    # chunk at row j of group g is 5j+g
    xa = np.empty((NCORES, 128, NGRP * XW), f8)
    cb = np.empty((NCORES, 128, NGRP * CW), f8)
    for g in range(NGRP):
        for j in range(GRP):
            ch = slice(500 * (5 * j + g), 500 * (5 * j + g) + 500)
            xa[:, :, XW * g + 500 * j:XW * g + 500 * (j + 1)] = xt[:, :, ch]
            cb[:, :, CW * g + 500 * j:CW * g + 500 * j + 500] = \
                ctf[:, 0:128, ch]
            cb[:, :, CW * g + 2000 + 500 * j:CW * g + 2500 + 500 * j] = \
                ctf[:, 128:256, ch]
        for p in range(2):
            for s in range(2):
                ch = slice(500 * (5 * (2 * p + s) + g),
                           500 * (5 * (2 * p + s) + g) + 500)
                cb[:, 64 * s:64 * s + 64,
                   CW * g + 4000 + 500 * p:CW * g + 4500 + 500 * p] = \
                    ctf[:, 256:320, ch]
    w8, wb, w3 = _pack_consts(
        np.asarray(W1, np.float32), np.asarray(b1, np.float32),
        np.asarray(W2, np.float32), np.asarray(b2, np.float32),
        np.asarray(Wfc, np.float32), np.asarray(Wc1, np.float32),
        np.asarray(bc1, np.float32), np.asarray(Wc2, np.float32))
    return xa, cb, w8, wb, w3


def kernel(node_features, col_features, edge_index=None,
           W1=None, b1=None, W2=None, b2=None, Wfc=None, bfc=None,
           Wc1=None, bc1=None, Wc2=None, bc2=None, **_unused):
    from concourse.bass_utils import run_bass_kernel_spmd

    xa, cb, w8, wb, w3 = _prep_inputs(node_features, col_features,
                                      W1, b1, W2, b2, Wfc, Wc1, bc1, Wc2)
    nc = _get_compiled()
    in_maps = [{"xt": xa[i], "cl": cb[i], "w8": w8, "wb": wb, "w3": w3}
               for i in range(NCORES)]
    res = run_bass_kernel_spmd(nc, in_maps, core_ids=list(range(NCORES)))
    outs = np.stack([np.asarray(res.results[i]["out"], np.float32)
                     for i in range(NCORES)])
    # outs[i][32j+k, 500g+nn] = value for node 500*(5j+g)+nn, class k
    o = outs.reshape(NCORES, 4, 32, NGRP, CH)[:, :, 0:K]   # [i, j, k, g, nn]
    o = o.transpose(0, 1, 3, 4, 2)                         # [i, j, g, nn, k]
    out = o.reshape(NCORES, GPC, N, K).reshape(B, N * K)
    return np.ascontiguousarray(out)

